# revision 16
# baseline (speedup 1.0000x reference)
"""Trainium2 Bass kernel for nn_CausalAttention_48945447305794.

Sharding: 8 cores = (batch b in {0,1}) x (head-group g in {0..3}).
Each core handles one batch and one group of 4 heads (one depthwise-conv
kernel size, padded to 7 taps), computes LN + QKV projection + causal
depthwise conv + alibi causal attention + a partial output projection
(its 256 channels of w_out). Host sums the 4 partial outputs per batch.

Device math (per core):
  - LN over tokens (bn_stats), gamma folded into W_qkv on host, beta via
    per-partition add of W@beta on the raw qkv signal.
  - h transposed 128x128 via PE into h^T (d on partitions), f32r.
  - QKV = W-slices @ h^T as f32r matmuls; depthwise conv as 7
    PSUM-accumulated diag(w_tap) matmuls; conv bias via tensor_scalar add.
  - Attention computed as S^T tiles [j=128, i=512] = k^T.T @ q^T with the
    alibi bias decomposed exactly: row term slope*(j-512ic) enters as the
    per-partition bias of the Exp activation; column term -slope*fi enters
    as a K=2 augmented matmul (hi/lo split on the f32r 11-bit grid); the
    per-query remainder cancels in softmax normalization.
  - Causal mask: gpsimd affine_select zero-fill on diagonal-crossing tiles
    after exp.
  - O^T = v_plus.T @ A^T with a ones column appended to v giving the
    softmax denominator as PSUM row 64; normalized via reciprocal +
    partition_broadcast + multiply.
  - Partial out-projection with K=64 per-head matmuls.
"""

import os
import sys

for _p in ("/opt/trn_rl_repo",):
    if _p not in sys.path and os.path.isdir(_p):
        sys.path.insert(0, _p)

import numpy as np

import concourse.bass as bass
import concourse.mybir as mybir
import concourse.tile as tile
from concourse import bacc
from concourse.bass_utils import run_bass_kernel_spmd

F32 = mybir.dt.float32
F32R = mybir.dt.float32r
U32 = mybir.dt.uint32
AF = mybir.ActivationFunctionType

B, N, DIM = 2, 2048, 1024
HEADS, DH = 16, 64
G, HPG, CG = 4, 4, 256
NT = N // 128          # 16 token tiles
NIC = N // 512         # 4 i-chunks
SCALE = DH ** -0.5
ONES_F32_BITS = 0x3F800000

_CACHE = {}
LAST = {}


def _round_f32r(a):
    """Round to nearest on the f32r grid (11-bit mantissa), HW-verified."""
    a = np.asarray(a, np.float64)
    out = np.zeros_like(a)
    nz = a != 0
    s = np.exp2(np.floor(np.log2(np.abs(a[nz]))))
    out[nz] = np.round(a[nz] / s * 2048.0) * s / 2048.0
    return out.astype(np.float32)


def _emit(tc, aps, debug):
    nc = tc.nc
    import contextlib
    ctx = contextlib.ExitStack()
    with ctx:
        singles = ctx.enter_context(tc.tile_pool(name="singles", bufs=1))
        p_w = ctx.enter_context(tc.tile_pool(name="p_w", bufs=1))
        p_diag = ctx.enter_context(tc.tile_pool(name="p_diag", bufs=1))
        p_raw = ctx.enter_context(tc.tile_pool(name="p_raw", bufs=1))
        p_qkv = ctx.enter_context(tc.tile_pool(name="p_qkv", bufs=1))
        p_v = ctx.enter_context(tc.tile_pool(name="p_v", bufs=1))
        p_vt = ctx.enter_context(tc.tile_pool(name="p_vt", bufs=1))
        p_oT = ctx.enter_context(tc.tile_pool(name="p_oT", bufs=1))
        p_x = ctx.enter_context(tc.tile_pool(name="p_x", bufs=2))
        p_h = ctx.enter_context(tc.tile_pool(name="p_h", bufs=1))
        p_hT = ctx.enter_context(tc.tile_pool(name="p_hT", bufs=2))
        p_aT = ctx.enter_context(tc.tile_pool(name="p_aT", bufs=3))
        p_norm = ctx.enter_context(tc.tile_pool(name="p_norm", bufs=1))
        p_out = ctx.enter_context(tc.tile_pool(name="p_out", bufs=2))
        p_stat = ctx.enter_context(tc.tile_pool(name="p_stat", bufs=4))

        ps_tr = ctx.enter_context(tc.tile_pool(name="ps_tr", bufs=1, space="PSUM"))
        ps_mm = ctx.enter_context(tc.tile_pool(name="ps_mm", bufs=1, space="PSUM"))
        ps_s = ctx.enter_context(tc.tile_pool(name="ps_s", bufs=4, space="PSUM"))
        ps_o = ctx.enter_context(tc.tile_pool(name="ps_o", bufs=1, space="PSUM"))

        # ---- constants / small inputs
        identr = singles.tile([128, 128], F32R)
        nc.gpsimd.memset(identr.bitcast(U32), 0)
        nc.gpsimd.affine_select(
            out=identr, in_=identr, compare_op=mybir.AluOpType.not_equal,
            fill=1.0, base=0, pattern=[[-1, 128]], channel_multiplier=1)
        ones2 = singles.tile([2, 128], F32R)
        nc.vector.memset(ones2.bitcast(U32), ONES_F32_BITS)
        eps_t = singles.tile([128, 1], F32)
        nc.vector.memset(eps_t, 1e-5)
        cbias = singles.tile([128, 3, 2], F32)
        nc.sync.dma_start(out=cbias, in_=aps["cbias"])
        wbeta = singles.tile([128, 3, 2], F32)
        nc.sync.dma_start(out=wbeta, in_=aps["wbeta"])
        amaster = singles.tile([128, 4, 16], F32)
        nc.sync.dma_start(out=amaster, in_=aps["amaster"])
        brows = singles.tile([2, 4, 512], F32R)
        nc.sync.dma_start(out=brows, in_=aps["brows"])

        wqkv = p_w.tile([128, 8, 768], F32R)
        for kd in range(8):
            nc.sync.dma_start(out=wqkv[:, kd, :], in_=aps["wqkv"][:, kd, :])
        diag = p_diag.tile([128, 3, 2, 7, 128], F32R, tag="diag")
        for prm in range(3):
            nc.sync.dma_start(out=diag[:, prm], in_=aps["diag"][:, prm])

        raws = [p_raw.tile([128, 2, 520], F32R, tag=f"raw{i}", name=f"raw{i}")
                for i in range(3)]
        for r in raws:
            nc.vector.memset(r[:, :, 0:8].bitcast(U32), 0)
        q_pairs = p_qkv.tile([128, 2, 2048], F32R, tag="q_pairs")
        k_pairs = p_qkv.tile([128, 2, 2048], F32R, tag="k_pairs")
        v_T = p_vt.tile([128, 4, 16, 65], F32R)
        nc.vector.memset(v_T[:, :, :, 64:65].bitcast(U32), ONES_F32_BITS)

        # ---- phase 1+2: LN + transpose + QKV + conv + v-transpose, per i-chunk
        for ic in range(NIC):
            hT = p_hT.tile([128, 8, 512], F32R, tag="hT")
            for t in range(4):
                nt = 4 * ic + t
                xt = p_x.tile([128, DIM], F32, tag="x")
                nc.sync.dma_start(out=xt, in_=aps["x_b"][nt * 128:(nt + 1) * 128, :])
                stats = p_stat.tile([128, 2, 6], F32, tag="stats")
                xg = xt.rearrange("p (s d) -> p s d", s=2)
                for sgi in range(2):
                    nc.vector.bn_stats(out=stats[:, sgi, :], in_=xg[:, sgi, :])
                mv = p_stat.tile([128, 2], F32, tag="mv")
                nc.vector.bn_aggr(out=mv, in_=stats)
                nc.scalar.activation(out=mv[:, 1:2], in_=mv[:, 1:2], func=AF.Sqrt,
                                     bias=eps_t, scale=1.0)
                nc.vector.reciprocal(out=mv[:, 1:2], in_=mv[:, 1:2])
                ht = p_h.tile([128, DIM], F32R, tag="h")
                nc.vector.tensor_scalar(
                    out=ht, in0=xt, scalar1=mv[:, 0:1], scalar2=mv[:, 1:2],
                    op0=mybir.AluOpType.subtract, op1=mybir.AluOpType.mult)
                for kd in range(8):
                    ptr = ps_tr.tile([128, 128], F32R, tag="tr")
                    nc.tensor.transpose(ptr, ht[:, kd * 128:(kd + 1) * 128], identr)
                    nc.vector.tensor_copy(out=hT[:, kd, t * 128:(t + 1) * 128], in_=ptr)
            vc = p_v.tile([128, 2, 512], F32R, tag="vc")
            for prm in range(3):
                for cc in range(2):
                    pmm = ps_mm.tile([128, 512], F32, tag="mm")
                    cbase = prm * 256 + cc * 128
                    for kd in range(8):
                        nc.tensor.matmul(pmm, wqkv[:, kd, cbase:cbase + 128],
                                         hT[:, kd, :], start=(kd == 0), stop=(kd == 7))
                    nc.vector.tensor_scalar(
                        out=raws[prm][:, cc, 8:520],
                        in0=pmm, scalar1=wbeta[:, prm, cc:cc + 1], scalar2=None,
                        op0=mybir.AluOpType.add)
                for cc in range(2):
                    pmm = ps_mm.tile([128, 512], F32, tag="mm")
                    for s in range(7):
                        nc.tensor.matmul(pmm, diag[:, prm, cc, s, :],
                                         raws[prm][:, cc, 2 + s: 2 + s + 512],
                                         start=(s == 0), stop=(s == 6))
                    if prm < 2:
                        dest = (q_pairs, k_pairs)[prm]
                        nc.vector.tensor_scalar(
                            out=dest[:, cc, ic * 512:(ic + 1) * 512],
                            in0=pmm, scalar1=cbias[:, prm, cc:cc + 1], scalar2=None,
                            op0=mybir.AluOpType.add)
                    else:
                        nc.vector.tensor_scalar(
                            out=vc[:, cc, :],
                            in0=pmm, scalar1=cbias[:, prm, cc:cc + 1], scalar2=None,
                            op0=mybir.AluOpType.add)
                # roll the 8-column tail for the next chunk's left taps
                tail = p_stat.tile([128, 2, 8], F32R, tag="tail")
                nc.vector.tensor_copy(out=tail, in_=raws[prm][:, :, 512:520])
                nc.vector.tensor_copy(out=raws[prm][:, :, 0:8], in_=tail)
            # v transposes for this chunk
            for cc in range(2):
                for t in range(4):
                    jt = 4 * ic + t
                    ptr = ps_tr.tile([128, 128], F32R, tag="tr")
                    nc.tensor.transpose(ptr, vc[:, cc, t * 128:(t + 1) * 128], identr)
                    nc.vector.tensor_copy(out=v_T[:, 2 * cc, jt, 0:64],
                                          in_=ptr[:, 0:64])
                    nc.vector.tensor_copy(out=v_T[:, 2 * cc + 1, jt, 0:64],
                                          in_=ptr[:, 64:128])
            if debug:
                nc.sync.dma_start(out=aps["dbg_v"][:, :, ic * 512:(ic + 1) * 512],
                                  in_=vc)

        # ---- phase 4+5: attention + partial out-projection, per i-chunk
        wout = p_w.tile([64, 4, 1024], F32R)
        nc.sync.dma_start(out=wout, in_=aps["wout"])
        for ic in range(NIC):
            o_T = p_oT.tile([64, 4, 512], F32R, tag="oT")
            njt = 4 * ic + 4              # j tiles 0 .. 4ic+3
            for hc in range(2):
                pos = [ps_o.tile([65, 512], F32, tag=f"po{i}", name=f"po{i}")
                       for i in range(2)]
                for jt in range(njt):
                    for hp in range(2):   # head pair: disjoint PE row groups
                        h = 2 * hc + hp
                        pb = 64 * hp
                        pS = ps_s.tile([128, 512], F32, tag="pS", name="pS")
                        nc.tensor.matmul(pS, ones2, brows[:, h, :],
                                         start=True, stop=False)
                        nc.tensor.matmul(
                            pS,
                            k_pairs[pb:pb + 64, hc, jt * 128:(jt + 1) * 128],
                            q_pairs[pb:pb + 64, hc, ic * 512:(ic + 1) * 512],
                            start=False, stop=True)
                        aT = p_aT.tile([128, 512], F32R, tag="aT")
                        m = jt - 4 * ic + 12
                        nc.scalar.activation(out=aT, in_=pS, func=AF.Exp,
                                             bias=amaster[:, h, m:m + 1], scale=1.0)
                        if jt >= 4 * ic:
                            nc.gpsimd.affine_select(
                                out=aT, in_=aT, compare_op=mybir.AluOpType.is_ge,
                                fill=0.0, base=512 * ic - 128 * jt,
                                pattern=[[1, 512]], channel_multiplier=-1)
                        nc.tensor.matmul(pos[hp], v_T[:, h, jt, :], aT,
                                         start=(jt == 0), stop=(jt == njt - 1))
                for hp in range(2):
                    h = 2 * hc + hp
                    scr = p_norm.tile([128, 512], F32, tag="scr")
                    nc.vector.reciprocal(out=scr[64:65, :], in_=pos[hp][64:65, :])
                    scr0 = p_norm.tile([1, 512], F32, tag="scr0")
                    nc.sync.dma_start(out=scr0, in_=scr[64:65, :])
                    bc = p_norm.tile([64, 512], F32, tag="bc")
                    nc.gpsimd.partition_broadcast(bc[:, :], scr0[:, :])
                    nc.vector.tensor_tensor(
                        out=o_T[0:64, h, :],
                        in0=pos[hp][0:64, :], in1=bc[:, :], op=mybir.AluOpType.mult)
            if debug:
                nc.sync.dma_start(out=aps["dbg_oT"][:, :, ic * 512:(ic + 1) * 512],
                                  in_=o_T)
            for t in range(4):
                nt = 4 * ic + t
                for ec in range(2):
                    pmm = ps_mm.tile([128, 512], F32, tag="mm")
                    for h in range(4):
                        nc.tensor.matmul(pmm, o_T[0:64, h, t * 128:(t + 1) * 128],
                                         wout[0:64, h, ec * 512:(ec + 1) * 512],
                                         start=(h == 0), stop=(h == 3))
                    ot = p_out.tile([128, 512], F32, tag="ot")
                    nc.vector.tensor_copy(out=ot, in_=pmm)
                    nc.sync.dma_start(
                        out=aps["out_partial"][nt * 128:(nt + 1) * 128,
                                               ec * 512:(ec + 1) * 512],
                        in_=ot)

        if debug:
            nc.sync.dma_start(out=aps["dbg_q"], in_=q_pairs)
            nc.sync.dma_start(out=aps["dbg_k"], in_=k_pairs)


def _build(debug=False):
    nc = bacc.Bacc("TRN2", target_bir_lowering=False, debug=False, num_devices=8)
    aps = {}
    def din(name, shape, dt):
        aps[name] = nc.dram_tensor(name, shape, dt, kind="ExternalInput").ap()
    din("x_b", [N, DIM], F32)
    din("wqkv", [128, 8, 768], F32R)
    din("diag", [128, 3, 2, 7, 128], F32R)
    din("cbias", [128, 3, 2], F32)
    din("wbeta", [128, 3, 2], F32)
    din("brows", [2, 4, 512], F32R)
    din("amaster", [128, 4, 16], F32)
    din("wout", [64, 4, 1024], F32R)
    aps["out_partial"] = nc.dram_tensor(
        "out_partial", [N, DIM], F32, kind="ExternalOutput").ap()
    if debug:
        for nm, shp in (("dbg_q", [128, 2, 2048]), ("dbg_k", [128, 2, 2048]),
                        ("dbg_v", [128, 2, 2048]), ("dbg_oT", [64, 4, 2048])):
            aps[nm] = nc.dram_tensor(nm, shp, F32R, kind="ExternalOutput").ap()
    with tile.TileContext(nc) as tc:
        _emit(tc, aps, debug)
    nc.compile()
    return nc


def _prepare_core(b, g, x, ln_gamma, ln_beta, w_qkv, dw, slopes, w_out):
    W_eff = w_qkv * ln_gamma[None, :]
    m = {}
    m["x_b"] = np.ascontiguousarray(x[b], dtype=np.float32)

    wqkv_sb = np.empty((128, 8, 768), np.float32)
    for prm in range(3):
        Wb = W_eff[prm * DIM + g * CG: prm * DIM + (g + 1) * CG]     # (256, 1024)
        wqkv_sb[:, :, prm * 256:(prm + 1) * 256] = (
            Wb.T.reshape(8, 128, 256).transpose(1, 0, 2))
    m["wqkv"] = np.ascontiguousarray(wqkv_sb)

    ksz = (0, 3, 5, 7)[g]
    w7 = np.zeros((3, 256, 7), np.float32)
    b7 = np.zeros((3, 256), np.float32)
    if ksz == 0:
        w7[:, :, 6] = 1.0
    else:
        for prm, p in enumerate("qkv"):
            w7[prm, :, 7 - ksz:] = dw[f"dw{p}_w{ksz}"][:, 0, :]
            b7[prm] = dw[f"dw{p}_b{ksz}"]
    w7[0] *= SCALE
    b7[0] *= SCALE

    dg = np.zeros((128, 3, 2, 7, 128), np.float32)
    pidx = np.arange(128)
    for prm in range(3):
        for cc in range(2):
            dg[pidx, prm, cc, :, pidx] = w7[prm, cc * 128: (cc + 1) * 128, :]
    m["diag"] = dg
    m["cbias"] = np.ascontiguousarray(
        b7.reshape(3, 2, 128).transpose(2, 0, 1))
    wb = np.stack([w_qkv[prm * DIM + g * CG: prm * DIM + (g + 1) * CG] @ ln_beta
                   for prm in range(3)])                             # (3, 256)
    m["wbeta"] = np.ascontiguousarray(
        wb.reshape(3, 2, 128).transpose(2, 0, 1)).astype(np.float32)

    sl = np.asarray(slopes, np.float64).reshape(-1)[4 * g: 4 * g + 4]
    Bm = -sl[:, None] * np.arange(512)[None, :]                      # (4, 512)
    B_hi = _round_f32r(Bm)
    B_lo = (Bm - B_hi).astype(np.float32)
    m["brows"] = np.ascontiguousarray(np.stack([B_hi, B_lo]))        # (2, 4, 512)

    am = np.empty((128, 4, 16), np.float32)
    p = np.arange(128)[:, None]
    mm = np.arange(16)[None, :]
    for h in range(4):
        am[:, h, :] = sl[h] * (p + 128.0 * (mm - 12))
    m["amaster"] = am

    WoT = w_out[:, g * CG:(g + 1) * CG].T                            # (256, 1024)
    m["wout"] = np.ascontiguousarray(WoT.reshape(4, 64, 1024).transpose(1, 0, 2))
    return m


def kernel(**inputs):
    key = "dbg" if os.environ.get("KBENCH_DEBUG") else "main"
    if key not in _CACHE:
        _CACHE[key] = _build(debug=(key == "dbg"))
    nc = _CACHE[key]

    x = np.asarray(inputs["x"], np.float32)
    ln_gamma = np.asarray(inputs["ln_gamma"], np.float32)
    ln_beta = np.asarray(inputs["ln_beta"], np.float32)
    w_qkv = np.asarray(inputs["w_qkv"], np.float32)
    slopes = np.asarray(inputs["slopes"], np.float32)
    w_out = np.asarray(inputs["w_out"], np.float32)
    dw = {k: np.asarray(v, np.float32) for k, v in inputs.items()
          if k.startswith("dw")}

    in_maps = [
        _prepare_core(c // 4, c % 4, x, ln_gamma, ln_beta, w_qkv, dw, slopes, w_out)
        for c in range(8)
    ]
    res = run_bass_kernel_spmd(
        nc, in_maps, list(range(8)),
        trace=bool(os.environ.get("KBENCH_TRACE")))
    LAST["res"] = res
    outs = [r["out_partial"] for r in res.results]
    out = np.stack([outs[0] + outs[1] + outs[2] + outs[3],
                    outs[4] + outs[5] + outs[6] + outs[7]], axis=0)
    return out.astype(np.float32)


# revision 19
# speedup vs baseline: 1.5656x; 1.5656x over previous
"""Trainium2 Bass kernel for nn_CausalAttention_48945447305794.

Sharding: 8 cores = (batch b in {0,1}) x (head-group g in {0..3}).
Each core handles one batch and one group of 4 heads (one depthwise-conv
kernel size, padded to 7 taps), computes LN + QKV projection + causal
depthwise conv + alibi causal attention + a partial output projection
(its 256 channels of w_out). Host sums the 4 partial outputs per batch.

All matmuls use float32r (HW-verified: round-to-nearest 11-bit mantissa,
1 cycle/row at free-dim>=256) and keep contraction K>=96: the PE clock
gate (HAM) never warms past 1.2 GHz for K<=66 matmuls, so small-K
contractions are zero-padded to K=128.

Device math (per core):
  - LN over tokens (bn_stats); gamma folded into W_qkv on host; beta via
    per-partition add of W@beta on the raw qkv signal.
  - h transposed 128x128 via PE into h^T (d on partitions), f32r.
  - QKV = W-slices @ h^T; depthwise conv as 7 PSUM-accumulated
    diag(w_tap) matmuls; conv bias added during the PSUM->SBUF copy.
  - q/k stored head-major [128, head, n]: rows 0-63 head data, rows
    64-65 alibi aug rows (q side: -slope*fi hi/lo split on the f32r
    grid; k side: ones), rows 66-127 zeros. One K=128 matmul per S^T
    tile [j=128, i=512] then computes q.k + alibi column term exactly.
  - Alibi row term slope*(j-512ic) enters as the per-partition bias of
    the Exp activation; the per-query remainder cancels in softmax
    normalization. Causal mask: gpsimd affine_select zero-fill on
    diagonal-crossing tiles after exp.
  - O^T = v_plus.T @ A^T with a ones column appended to v giving the
    softmax denominator as PSUM row 64; PSUM copied out immediately,
    normalized off-PSUM via reciprocal + partition_broadcast + multiply.
  - Out-projection with heads paired for K=128 matmuls.
"""

import os
import sys

for _p in ("/opt/trn_rl_repo",):
    if _p not in sys.path and os.path.isdir(_p):
        sys.path.insert(0, _p)

import numpy as np

import concourse.bass as bass
import concourse.mybir as mybir
import concourse.tile as tile
from concourse import bacc
from concourse.bass_utils import run_bass_kernel_spmd

F32 = mybir.dt.float32
F32R = mybir.dt.float32r
U32 = mybir.dt.uint32
AF = mybir.ActivationFunctionType

B, N, DIM = 2, 2048, 1024
HEADS, DH = 16, 64
G, HPG, CG = 4, 4, 256
NT = N // 128          # 16 token tiles
NIC = N // 512         # 4 i-chunks
SCALE = DH ** -0.5
ONES_F32_BITS = 0x3F800000

_CACHE = {}
LAST = {}


def _round_f32r(a):
    """Round to nearest on the f32r grid (11-bit mantissa), HW-verified."""
    a = np.asarray(a, np.float64)
    out = np.zeros_like(a)
    nz = a != 0
    s = np.exp2(np.floor(np.log2(np.abs(a[nz]))))
    out[nz] = np.round(a[nz] / s * 2048.0) * s / 2048.0
    return out.astype(np.float32)


def _emit(tc, aps, debug):
    nc = tc.nc
    import contextlib
    ctx = contextlib.ExitStack()
    with ctx:
        singles = ctx.enter_context(tc.tile_pool(name="singles", bufs=1))
        p_w = ctx.enter_context(tc.tile_pool(name="p_w", bufs=1))
        p_diag = ctx.enter_context(tc.tile_pool(name="p_diag", bufs=1))
        p_raw = ctx.enter_context(tc.tile_pool(name="p_raw", bufs=1))
        p_qkv = ctx.enter_context(tc.tile_pool(name="p_qkv", bufs=1))
        p_v = ctx.enter_context(tc.tile_pool(name="p_v", bufs=1))
        p_vt = ctx.enter_context(tc.tile_pool(name="p_vt", bufs=1))
        p_oT = ctx.enter_context(tc.tile_pool(name="p_oT", bufs=1))
        p_x = ctx.enter_context(tc.tile_pool(name="p_x", bufs=2))
        p_h = ctx.enter_context(tc.tile_pool(name="p_h", bufs=1))
        p_hT = ctx.enter_context(tc.tile_pool(name="p_hT", bufs=1))
        p_aT = ctx.enter_context(tc.tile_pool(name="p_aT", bufs=3))
        p_norm = ctx.enter_context(tc.tile_pool(name="p_norm", bufs=1))
        p_scr = ctx.enter_context(tc.tile_pool(name="p_scr", bufs=2))
        p_out = ctx.enter_context(tc.tile_pool(name="p_out", bufs=2))
        p_stat = ctx.enter_context(tc.tile_pool(name="p_stat", bufs=4))

        ps_tr = ctx.enter_context(tc.tile_pool(name="ps_tr", bufs=1, space="PSUM"))
        ps_mm = ctx.enter_context(tc.tile_pool(name="ps_mm", bufs=2, space="PSUM"))
        ps_s = ctx.enter_context(tc.tile_pool(name="ps_s", bufs=3, space="PSUM"))
        ps_o = ctx.enter_context(tc.tile_pool(name="ps_o", bufs=1, space="PSUM"))

        # ---- constants / small inputs
        identr = singles.tile([128, 128], F32R)
        nc.gpsimd.memset(identr.bitcast(U32), 0)
        nc.gpsimd.affine_select(
            out=identr, in_=identr, compare_op=mybir.AluOpType.not_equal,
            fill=1.0, base=0, pattern=[[-1, 128]], channel_multiplier=1)
        eps_t = singles.tile([128, 1], F32)
        nc.vector.memset(eps_t, 1e-5)
        cbias = singles.tile([128, 3, 2], F32)
        nc.sync.dma_start(out=cbias, in_=aps["cbias"])
        wbeta = singles.tile([128, 3, 2], F32)
        nc.sync.dma_start(out=wbeta, in_=aps["wbeta"])
        amaster = singles.tile([128, 4, 16], F32)
        nc.sync.dma_start(out=amaster, in_=aps["amaster"])

        wqkv = p_w.tile([128, 8, 768], F32R)
        for kd in range(8):
            nc.sync.dma_start(out=wqkv[:, kd, :], in_=aps["wqkv"][:, kd, :])
        diag = p_diag.tile([128, 3, 2, 7, 128], F32R, tag="diag")
        for prm in range(3):
            nc.sync.dma_start(out=diag[:, prm], in_=aps["diag"][:, prm])

        raws = [p_raw.tile([128, 2, 520], F32R, tag=f"raw{i}", name=f"raw{i}")
                for i in range(3)]
        for r in raws:
            nc.vector.memset(r[:, :, 0:8].bitcast(U32), 0)

        # q/k head-major with aug rows; rows 66-127 must be zero
        q_aug = p_qkv.tile([128, 4, 2048], F32R, tag="q_aug")
        k_aug = p_qkv.tile([128, 4, 2048], F32R, tag="k_aug")
        nc.vector.memset(q_aug[64:128].bitcast(U32), 0)
        nc.vector.memset(k_aug[64:128].bitcast(U32), 0)
        nc.sync.dma_start(out=q_aug[64:66], in_=aps["brows"])
        nc.vector.memset(k_aug[64:66].bitcast(U32), ONES_F32_BITS)

        v_T = p_vt.tile([128, 4, 16, 65], F32R)
        nc.vector.memset(v_T[:, :, :, 64:65].bitcast(U32), ONES_F32_BITS)

        # ---- phase 1+2: LN + transpose + QKV + conv + v-transpose, per chunk
        for ic in range(NIC):
            hT = p_hT.tile([128, 8, 512], F32R, tag="hT")
            for t in range(4):
                nt = 4 * ic + t
                xt = p_x.tile([128, DIM], F32, tag="x")
                nc.sync.dma_start(out=xt, in_=aps["x_b"][nt * 128:(nt + 1) * 128, :])
                stats = p_stat.tile([128, 2, 6], F32, tag="stats")
                xg = xt.rearrange("p (s d) -> p s d", s=2)
                for sgi in range(2):
                    nc.vector.bn_stats(out=stats[:, sgi, :], in_=xg[:, sgi, :])
                mv = p_stat.tile([128, 2], F32, tag="mv")
                nc.vector.bn_aggr(out=mv, in_=stats)
                nc.scalar.activation(out=mv[:, 1:2], in_=mv[:, 1:2], func=AF.Sqrt,
                                     bias=eps_t, scale=1.0)
                nc.vector.reciprocal(out=mv[:, 1:2], in_=mv[:, 1:2])
                ht = p_h.tile([128, DIM], F32R, tag="h")
                nc.vector.tensor_scalar(
                    out=ht, in0=xt, scalar1=mv[:, 0:1], scalar2=mv[:, 1:2],
                    op0=mybir.AluOpType.subtract, op1=mybir.AluOpType.mult)
                for kd in range(8):
                    ptr = ps_tr.tile([128, 128], F32R, tag="tr")
                    nc.tensor.transpose(ptr, ht[:, kd * 128:(kd + 1) * 128], identr)
                    nc.vector.tensor_copy(out=hT[:, kd, t * 128:(t + 1) * 128], in_=ptr)
            vc = p_v.tile([128, 2, 512], F32R, tag="vc")
            for prm in range(3):
                for cc in range(2):
                    pmm = ps_mm.tile([128, 512], F32, tag="mm")
                    cbase = prm * 256 + cc * 128
                    for kd in range(8):
                        nc.tensor.matmul(pmm, wqkv[:, kd, cbase:cbase + 128],
                                         hT[:, kd, :], start=(kd == 0), stop=(kd == 7))
                    nc.vector.tensor_scalar(
                        out=raws[prm][:, cc, 8:520],
                        in0=pmm, scalar1=wbeta[:, prm, cc:cc + 1], scalar2=None,
                        op0=mybir.AluOpType.add)
                for cc in range(2):
                    pmm = ps_mm.tile([128, 512], F32, tag="mm")
                    for s in range(7):
                        nc.tensor.matmul(pmm, diag[:, prm, cc, s, :],
                                         raws[prm][:, cc, 2 + s: 2 + s + 512],
                                         start=(s == 0), stop=(s == 6))
                    isl = slice(ic * 512, (ic + 1) * 512)
                    if prm < 2:
                        dest = (q_aug, k_aug)[prm]
                        # even head of the pair: aligned copy
                        nc.vector.tensor_scalar(
                            out=dest[0:64, 2 * cc, isl], in0=pmm[0:64, :],
                            scalar1=cbias[0:64, prm, cc:cc + 1], scalar2=None,
                            op0=mybir.AluOpType.add)
                        # odd head: bias-add into scratch, DMA partition-shift
                        scrA = p_scr.tile([128, 512], F32R, tag="scrA")
                        nc.vector.tensor_scalar(
                            out=scrA[64:128, :], in0=pmm[64:128, :],
                            scalar1=cbias[64:128, prm, cc:cc + 1], scalar2=None,
                            op0=mybir.AluOpType.add)
                        nc.sync.dma_start(out=dest[0:64, 2 * cc + 1, isl],
                                          in_=scrA[64:128, :])
                    else:
                        nc.vector.tensor_scalar(
                            out=vc[:, cc, :],
                            in0=pmm, scalar1=cbias[:, prm, cc:cc + 1], scalar2=None,
                            op0=mybir.AluOpType.add)
                # roll the 8-column tail for the next chunk's left taps
                tail = p_stat.tile([128, 2, 8], F32R, tag="tail")
                nc.vector.tensor_copy(out=tail, in_=raws[prm][:, :, 512:520])
                nc.vector.tensor_copy(out=raws[prm][:, :, 0:8], in_=tail)
            # v transposes for this chunk
            for cc in range(2):
                for t in range(4):
                    jt = 4 * ic + t
                    ptr = ps_tr.tile([128, 128], F32R, tag="tr")
                    nc.tensor.transpose(ptr, vc[:, cc, t * 128:(t + 1) * 128], identr)
                    nc.vector.tensor_copy(out=v_T[:, 2 * cc, jt, 0:64],
                                          in_=ptr[:, 0:64])
                    nc.vector.tensor_copy(out=v_T[:, 2 * cc + 1, jt, 0:64],
                                          in_=ptr[:, 64:128])
            if debug:
                nc.sync.dma_start(out=aps["dbg_v"][:, :, ic * 512:(ic + 1) * 512],
                                  in_=vc)

        # ---- phase 4+5: attention + partial out-projection, per i-chunk
        wout = p_w.tile([128, 2, 1024], F32R)
        nc.sync.dma_start(out=wout, in_=aps["wout"])
        for ic in range(NIC):
            o_pair = p_oT.tile([128, 2, 512], F32R, tag="oP")
            njt = 4 * ic + 4              # j tiles 0 .. 4ic+3
            for hc in range(2):
                pos = [ps_o.tile([65, 512], F32, tag=f"po{i}", name=f"po{i}")
                       for i in range(2)]
                for jt in range(njt):
                    for hp in range(2):
                        h = 2 * hc + hp
                        pS = ps_s.tile([128, 512], F32, tag="pS", name="pS")
                        nc.tensor.matmul(
                            pS,
                            k_aug[:, h, jt * 128:(jt + 1) * 128],
                            q_aug[:, h, ic * 512:(ic + 1) * 512],
                            start=True, stop=True)
                        aT = p_aT.tile([128, 512], F32R, tag="aT")
                        m = jt - 4 * ic + 12
                        nc.scalar.activation(out=aT, in_=pS, func=AF.Exp,
                                             bias=amaster[:, h, m:m + 1], scale=1.0)
                        if jt >= 4 * ic:
                            nc.gpsimd.affine_select(
                                out=aT, in_=aT, compare_op=mybir.AluOpType.is_ge,
                                fill=0.0, base=512 * ic - 128 * jt,
                                pattern=[[1, 512]], channel_multiplier=-1)
                        nc.tensor.matmul(pos[hp], v_T[:, h, jt, :], aT,
                                         start=(jt == 0), stop=(jt == njt - 1))
                for hp in range(2):
                    # free the PSUM bank first, normalize off-PSUM
                    otmp = p_norm.tile([65, 512], F32, tag="otmp")
                    nc.vector.tensor_copy(out=otmp, in_=pos[hp])
                    nc.vector.reciprocal(out=otmp[64:65, :], in_=otmp[64:65, :])
                    scr0 = p_norm.tile([1, 512], F32, tag="scr0")
                    nc.sync.dma_start(out=scr0, in_=otmp[64:65, :])
                    bc = p_norm.tile([64, 512], F32, tag="bc")
                    nc.gpsimd.partition_broadcast(bc[:, :], scr0[:, :])
                    if hp == 0:
                        nc.vector.tensor_tensor(
                            out=o_pair[0:64, hc, :], in0=otmp[0:64, :],
                            in1=bc[:, :], op=mybir.AluOpType.mult)
                    else:
                        scr3 = p_norm.tile([64, 512], F32R, tag="scr3")
                        nc.vector.tensor_tensor(
                            out=scr3, in0=otmp[0:64, :],
                            in1=bc[:, :], op=mybir.AluOpType.mult)
                        nc.sync.dma_start(out=o_pair[64:128, hc, :], in_=scr3)
            if debug:
                nc.sync.dma_start(out=aps["dbg_oP"][:, :, ic * 512:(ic + 1) * 512],
                                  in_=o_pair)
            for t in range(4):
                nt = 4 * ic + t
                for ec in range(2):
                    pmm = ps_mm.tile([128, 512], F32, tag="mm")
                    for hc in range(2):
                        nc.tensor.matmul(pmm, o_pair[:, hc, t * 128:(t + 1) * 128],
                                         wout[:, hc, ec * 512:(ec + 1) * 512],
                                         start=(hc == 0), stop=(hc == 1))
                    ot = p_out.tile([128, 512], F32, tag="ot")
                    nc.vector.tensor_copy(out=ot, in_=pmm)
                    nc.sync.dma_start(
                        out=aps["out_partial"][nt * 128:(nt + 1) * 128,
                                               ec * 512:(ec + 1) * 512],
                        in_=ot)

        if debug:
            nc.sync.dma_start(out=aps["dbg_q"], in_=q_aug)
            nc.sync.dma_start(out=aps["dbg_k"], in_=k_aug)


def _build(debug=False):
    nc = bacc.Bacc("TRN2", target_bir_lowering=False, debug=False, num_devices=8)
    aps = {}
    def din(name, shape, dt):
        aps[name] = nc.dram_tensor(name, shape, dt, kind="ExternalInput").ap()
    din("x_b", [N, DIM], F32)
    din("wqkv", [128, 8, 768], F32R)
    din("diag", [128, 3, 2, 7, 128], F32R)
    din("cbias", [128, 3, 2], F32)
    din("wbeta", [128, 3, 2], F32)
    din("brows", [2, 4, 2048], F32R)
    din("amaster", [128, 4, 16], F32)
    din("wout", [128, 2, 1024], F32R)
    aps["out_partial"] = nc.dram_tensor(
        "out_partial", [N, DIM], F32, kind="ExternalOutput").ap()
    if debug:
        for nm, shp in (("dbg_q", [128, 4, 2048]), ("dbg_k", [128, 4, 2048]),
                        ("dbg_v", [128, 2, 2048]), ("dbg_oP", [128, 2, 2048])):
            aps[nm] = nc.dram_tensor(nm, shp, F32R, kind="ExternalOutput").ap()
    with tile.TileContext(nc) as tc:
        _emit(tc, aps, debug)
    nc.compile()
    return nc


def _prepare_core(b, g, x, ln_gamma, ln_beta, w_qkv, dw, slopes, w_out):
    W_eff = w_qkv * ln_gamma[None, :]
    m = {}
    m["x_b"] = np.ascontiguousarray(x[b], dtype=np.float32)

    wqkv_sb = np.empty((128, 8, 768), np.float32)
    for prm in range(3):
        Wb = W_eff[prm * DIM + g * CG: prm * DIM + (g + 1) * CG]     # (256, 1024)
        wqkv_sb[:, :, prm * 256:(prm + 1) * 256] = (
            Wb.T.reshape(8, 128, 256).transpose(1, 0, 2))
    m["wqkv"] = np.ascontiguousarray(wqkv_sb)

    ksz = (0, 3, 5, 7)[g]
    w7 = np.zeros((3, 256, 7), np.float32)
    b7 = np.zeros((3, 256), np.float32)
    if ksz == 0:
        w7[:, :, 6] = 1.0
    else:
        for prm, p in enumerate("qkv"):
            w7[prm, :, 7 - ksz:] = dw[f"dw{p}_w{ksz}"][:, 0, :]
            b7[prm] = dw[f"dw{p}_b{ksz}"]
    w7[0] *= SCALE
    b7[0] *= SCALE

    dg = np.zeros((128, 3, 2, 7, 128), np.float32)
    pidx = np.arange(128)
    for prm in range(3):
        for cc in range(2):
            dg[pidx, prm, cc, :, pidx] = w7[prm, cc * 128: (cc + 1) * 128, :]
    m["diag"] = dg
    m["cbias"] = np.ascontiguousarray(
        b7.reshape(3, 2, 128).transpose(2, 0, 1))
    wb = np.stack([w_qkv[prm * DIM + g * CG: prm * DIM + (g + 1) * CG] @ ln_beta
                   for prm in range(3)])                             # (3, 256)
    m["wbeta"] = np.ascontiguousarray(
        wb.reshape(3, 2, 128).transpose(2, 0, 1)).astype(np.float32)

    sl = np.asarray(slopes, np.float64).reshape(-1)[4 * g: 4 * g + 4]
    Bm = -sl[:, None] * np.arange(512)[None, :]                      # (4, 512)
    B_hi = _round_f32r(Bm)
    B_lo = (Bm - B_hi).astype(np.float32)
    m["brows"] = np.ascontiguousarray(
        np.tile(np.stack([B_hi, B_lo]), (1, 1, 4)))                  # (2, 4, 2048)

    am = np.empty((128, 4, 16), np.float32)
    p = np.arange(128)[:, None]
    mm = np.arange(16)[None, :]
    for h in range(4):
        am[:, h, :] = sl[h] * (p + 128.0 * (mm - 12))
    m["amaster"] = am

    WoT = w_out[:, g * CG:(g + 1) * CG].T                            # (256, 1024)
    m["wout"] = np.ascontiguousarray(WoT.reshape(2, 128, 1024).transpose(1, 0, 2))
    return m


def kernel(**inputs):
    key = "dbg" if os.environ.get("KBENCH_DEBUG") else "main"
    if key not in _CACHE:
        _CACHE[key] = _build(debug=(key == "dbg"))
    nc = _CACHE[key]

    x = np.asarray(inputs["x"], np.float32)
    ln_gamma = np.asarray(inputs["ln_gamma"], np.float32)
    ln_beta = np.asarray(inputs["ln_beta"], np.float32)
    w_qkv = np.asarray(inputs["w_qkv"], np.float32)
    slopes = np.asarray(inputs["slopes"], np.float32)
    w_out = np.asarray(inputs["w_out"], np.float32)
    dw = {k: np.asarray(v, np.float32) for k, v in inputs.items()
          if k.startswith("dw")}

    in_maps = [
        _prepare_core(c // 4, c % 4, x, ln_gamma, ln_beta, w_qkv, dw, slopes, w_out)
        for c in range(8)
    ]
    res = run_bass_kernel_spmd(
        nc, in_maps, list(range(8)),
        trace=bool(os.environ.get("KBENCH_TRACE")))
    LAST["res"] = res
    outs = [r["out_partial"] for r in res.results]
    out = np.stack([outs[0] + outs[1] + outs[2] + outs[3],
                    outs[4] + outs[5] + outs[6] + outs[7]], axis=0)
    return out.astype(np.float32)


# revision 22
# speedup vs baseline: 1.6821x; 1.0744x over previous
"""Trainium2 Bass kernel for nn_CausalAttention_48945447305794.

Sharding: 8 cores = (batch b in {0,1}) x (head-group g in {0..3}).
Each core handles one batch and one group of 4 heads (one depthwise-conv
kernel size, padded to 7 taps), computes LN + QKV projection + causal
depthwise conv + alibi causal attention + a partial output projection
(its 256 channels of w_out). Host sums the 4 partial outputs per batch.

All matmuls use float32r (HW-verified: round-to-nearest 11-bit mantissa,
1 cycle/row at free-dim>=256) and keep contraction K>=96: the PE clock
gate (HAM) never warms past 1.2 GHz for K<=66 matmuls, so small-K
contractions are zero-padded to K=128.

Device math (per core):
  - LN over tokens (bn_stats); gamma folded into W_qkv on host; beta via
    per-partition add of W@beta on the raw qkv signal.
  - h transposed 128x128 via PE into h^T (d on partitions), f32r.
  - QKV = W-slices @ h^T; depthwise conv as 7 PSUM-accumulated
    diag(w_tap) matmuls; conv bias added during the PSUM->SBUF copy.
  - q/k stored head-major [128, head, n]: rows 0-63 head data, rows
    64-65 alibi aug rows (q side: -slope*fi hi/lo split on the f32r
    grid; k side: ones), rows 66-127 zeros. One K=128 matmul per S^T
    tile [j=128, i=512] then computes q.k + alibi column term exactly.
  - Alibi row term slope*(j-512ic) enters as the per-partition bias of
    the Exp activation; the per-query remainder cancels in softmax
    normalization. Causal mask: gpsimd affine_select zero-fill on
    diagonal-crossing tiles after exp.
  - O^T = v_plus.T @ A^T with a ones column appended to v giving the
    softmax denominator as PSUM row 64; PSUM copied out immediately,
    normalized off-PSUM via reciprocal + partition_broadcast + multiply.
  - Out-projection with heads paired for K=128 matmuls.
"""

import os
import sys

for _p in ("/opt/trn_rl_repo",):
    if _p not in sys.path and os.path.isdir(_p):
        sys.path.insert(0, _p)

import numpy as np

import concourse.bass as bass
import concourse.mybir as mybir
import concourse.tile as tile
from concourse import bacc
from concourse.bass_utils import run_bass_kernel_spmd

F32 = mybir.dt.float32
F32R = mybir.dt.float32r
U32 = mybir.dt.uint32
AF = mybir.ActivationFunctionType

B, N, DIM = 2, 2048, 1024
HEADS, DH = 16, 64
G, HPG, CG = 4, 4, 256
NT = N // 128          # 16 token tiles
NIC = N // 512         # 4 i-chunks
SCALE = DH ** -0.5
ONES_F32_BITS = 0x3F800000

_CACHE = {}
LAST = {}


def _round_f32r(a):
    """Round to nearest on the f32r grid (11-bit mantissa), HW-verified."""
    a = np.asarray(a, np.float64)
    out = np.zeros_like(a)
    nz = a != 0
    s = np.exp2(np.floor(np.log2(np.abs(a[nz]))))
    out[nz] = np.round(a[nz] / s * 2048.0) * s / 2048.0
    return out.astype(np.float32)


def _emit(tc, aps, debug):
    nc = tc.nc
    import contextlib
    ctx = contextlib.ExitStack()
    with ctx:
        singles = ctx.enter_context(tc.tile_pool(name="singles", bufs=1))
        p_w = ctx.enter_context(tc.tile_pool(name="p_w", bufs=1))
        p_diag = ctx.enter_context(tc.tile_pool(name="p_diag", bufs=1))
        p_raw = ctx.enter_context(tc.tile_pool(name="p_raw", bufs=1))
        p_qkv = ctx.enter_context(tc.tile_pool(name="p_qkv", bufs=1))
        p_v = ctx.enter_context(tc.tile_pool(name="p_v", bufs=1))
        p_vt = ctx.enter_context(tc.tile_pool(name="p_vt", bufs=1))
        p_oT = ctx.enter_context(tc.tile_pool(name="p_oT", bufs=1))
        p_x = ctx.enter_context(tc.tile_pool(name="p_x", bufs=2))
        p_h = ctx.enter_context(tc.tile_pool(name="p_h", bufs=1))
        p_hT = ctx.enter_context(tc.tile_pool(name="p_hT", bufs=1))
        p_aT = ctx.enter_context(tc.tile_pool(name="p_aT", bufs=3))
        p_norm = ctx.enter_context(tc.tile_pool(name="p_norm", bufs=1))
        p_scr = ctx.enter_context(tc.tile_pool(name="p_scr", bufs=2))
        p_out = ctx.enter_context(tc.tile_pool(name="p_out", bufs=2))
        p_stat = ctx.enter_context(tc.tile_pool(name="p_stat", bufs=4))

        ps_tr = ctx.enter_context(tc.tile_pool(name="ps_tr", bufs=1, space="PSUM"))
        ps_mm = ctx.enter_context(tc.tile_pool(name="ps_mm", bufs=2, space="PSUM"))
        ps_s = ctx.enter_context(tc.tile_pool(name="ps_s", bufs=3, space="PSUM"))
        ps_o = ctx.enter_context(tc.tile_pool(name="ps_o", bufs=1, space="PSUM"))

        # ---- constants / small inputs
        identr = singles.tile([128, 128], F32R)
        nc.gpsimd.memset(identr.bitcast(U32), 0)
        nc.gpsimd.affine_select(
            out=identr, in_=identr, compare_op=mybir.AluOpType.not_equal,
            fill=1.0, base=0, pattern=[[-1, 128]], channel_multiplier=1)
        eps_t = singles.tile([128, 1], F32)
        nc.vector.memset(eps_t, 1e-5)
        cbias = singles.tile([128, 3, 2], F32)
        nc.sync.dma_start(out=cbias, in_=aps["cbias"])
        wbeta = singles.tile([128, 3, 2], F32)
        nc.sync.dma_start(out=wbeta, in_=aps["wbeta"])
        amaster = singles.tile([128, 4, 16], F32)
        nc.sync.dma_start(out=amaster, in_=aps["amaster"])

        wqkv = p_w.tile([128, 8, 768], F32R)
        for kd in range(8):
            nc.gpsimd.dma_start(out=wqkv[:, kd, :], in_=aps["wqkv"][:, kd, :])
        diag = p_diag.tile([128, 3, 2, 7, 128], F32R, tag="diag")
        for prm in range(3):
            nc.gpsimd.dma_start(out=diag[:, prm], in_=aps["diag"][:, prm])

        raws = [p_raw.tile([128, 2, 520], F32R, tag=f"raw{i}", name=f"raw{i}")
                for i in range(3)]
        for r in raws:
            nc.vector.memset(r[:, :, 0:8].bitcast(U32), 0)

        # q/k head-major with aug rows; rows 66-127 must be zero
        q_aug = p_qkv.tile([128, 4, 2048], F32R, tag="q_aug")
        k_aug = p_qkv.tile([128, 4, 2048], F32R, tag="k_aug")
        nc.gpsimd.memset(q_aug[64:128].bitcast(U32), 0)
        nc.gpsimd.memset(k_aug[64:128].bitcast(U32), 0)
        nc.sync.dma_start(out=q_aug[64:66], in_=aps["brows"])
        nc.gpsimd.memset(k_aug[64:66].bitcast(U32), ONES_F32_BITS)

        v_T = p_vt.tile([128, 4, 16, 65], F32R)
        nc.gpsimd.memset(v_T[:, :, :, 64:65].bitcast(U32), ONES_F32_BITS)

        # ---- phase 1+2: LN + transpose + QKV + conv + v-transpose, per chunk
        for ic in range(NIC):
            hT = p_hT.tile([128, 8, 512], F32R, tag="hT")
            for t in range(4):
                nt = 4 * ic + t
                xt = p_x.tile([128, DIM], F32, tag="x")
                nc.sync.dma_start(out=xt, in_=aps["x_b"][nt * 128:(nt + 1) * 128, :])
                stats = p_stat.tile([128, 2, 6], F32, tag="stats")
                xg = xt.rearrange("p (s d) -> p s d", s=2)
                for sgi in range(2):
                    nc.vector.bn_stats(out=stats[:, sgi, :], in_=xg[:, sgi, :])
                mv = p_stat.tile([128, 2], F32, tag="mv")
                nc.vector.bn_aggr(out=mv, in_=stats)
                nc.scalar.activation(out=mv[:, 1:2], in_=mv[:, 1:2], func=AF.Sqrt,
                                     bias=eps_t, scale=1.0)
                nc.vector.reciprocal(out=mv[:, 1:2], in_=mv[:, 1:2])
                ht = p_h.tile([128, DIM], F32R, tag="h")
                nc.vector.tensor_scalar(
                    out=ht, in0=xt, scalar1=mv[:, 0:1], scalar2=mv[:, 1:2],
                    op0=mybir.AluOpType.subtract, op1=mybir.AluOpType.mult)
                for kq in range(2):
                    ptr = ps_tr.tile([128, 4, 128], F32R, tag="tr")
                    for kk in range(4):
                        kd = 4 * kq + kk
                        nc.tensor.transpose(ptr[:, kk, :],
                                            ht[:, kd * 128:(kd + 1) * 128], identr)
                    nc.vector.tensor_copy(
                        out=hT[:, 4 * kq:4 * kq + 4, t * 128:(t + 1) * 128],
                        in_=ptr)
            vc = p_v.tile([128, 2, 512], F32R, tag="vc")
            for prm in range(3):
                for cc in range(2):
                    pmm = ps_mm.tile([128, 512], F32, tag="mm")
                    cbase = prm * 256 + cc * 128
                    for kd in range(8):
                        nc.tensor.matmul(pmm, wqkv[:, kd, cbase:cbase + 128],
                                         hT[:, kd, :], start=(kd == 0), stop=(kd == 7))
                    nc.scalar.activation(
                        out=raws[prm][:, cc, 8:520], in_=pmm, func=AF.Identity,
                        bias=wbeta[:, prm, cc:cc + 1], scale=1.0)
                for cc in range(2):
                    pmm = ps_mm.tile([128, 512], F32, tag="mm")
                    for s in range(7):
                        nc.tensor.matmul(pmm, diag[:, prm, cc, s, :],
                                         raws[prm][:, cc, 2 + s: 2 + s + 512],
                                         start=(s == 0), stop=(s == 6))
                    isl = slice(ic * 512, (ic + 1) * 512)
                    if prm < 2:
                        dest = (q_aug, k_aug)[prm]
                        # even head of the pair: aligned copy
                        nc.vector.tensor_scalar(
                            out=dest[0:64, 2 * cc, isl], in0=pmm[0:64, :],
                            scalar1=cbias[0:64, prm, cc:cc + 1], scalar2=None,
                            op0=mybir.AluOpType.add)
                        # odd head: bias-add into scratch, DMA partition-shift
                        scrA = p_scr.tile([128, 512], F32R, tag="scrA")
                        nc.vector.tensor_scalar(
                            out=scrA[64:128, :], in0=pmm[64:128, :],
                            scalar1=cbias[64:128, prm, cc:cc + 1], scalar2=None,
                            op0=mybir.AluOpType.add)
                        nc.sync.dma_start(out=dest[0:64, 2 * cc + 1, isl],
                                          in_=scrA[64:128, :])
                    else:
                        nc.vector.tensor_scalar(
                            out=vc[:, cc, :],
                            in0=pmm, scalar1=cbias[:, prm, cc:cc + 1], scalar2=None,
                            op0=mybir.AluOpType.add)
                # roll the 8-column tail for the next chunk's left taps
                tail = p_stat.tile([128, 2, 8], F32R, tag="tail")
                nc.vector.tensor_copy(out=tail, in_=raws[prm][:, :, 512:520])
                nc.vector.tensor_copy(out=raws[prm][:, :, 0:8], in_=tail)
            # v transposes for this chunk
            for cc in range(2):
                for t in range(4):
                    jt = 4 * ic + t
                    ptr = ps_tr.tile([128, 128], F32R, tag="tr")
                    nc.tensor.transpose(ptr, vc[:, cc, t * 128:(t + 1) * 128], identr)
                    nc.vector.tensor_copy(out=v_T[:, 2 * cc, jt, 0:64],
                                          in_=ptr[:, 0:64])
                    nc.vector.tensor_copy(out=v_T[:, 2 * cc + 1, jt, 0:64],
                                          in_=ptr[:, 64:128])
            if debug:
                nc.sync.dma_start(out=aps["dbg_v"][:, :, ic * 512:(ic + 1) * 512],
                                  in_=vc)

        # ---- phase 4+5: attention + partial out-projection, per i-chunk
        wout = p_w.tile([128, 2, 1024], F32R)
        nc.gpsimd.dma_start(out=wout, in_=aps["wout"])
        for ic in range(NIC):
            o_pair = p_oT.tile([128, 2, 512], F32R, tag="oP")
            njt = 4 * ic + 4              # j tiles 0 .. 4ic+3
            for hc in range(2):
                pos = [ps_o.tile([65, 512], F32, tag=f"po{i}", name=f"po{i}")
                       for i in range(2)]
                for jt in range(njt):
                    for hp in range(2):
                        h = 2 * hc + hp
                        pS = ps_s.tile([128, 512], F32, tag="pS", name="pS")
                        nc.tensor.matmul(
                            pS,
                            k_aug[:, h, jt * 128:(jt + 1) * 128],
                            q_aug[:, h, ic * 512:(ic + 1) * 512],
                            start=True, stop=True)
                        aT = p_aT.tile([128, 512], F32R, tag="aT")
                        m = jt - 4 * ic + 12
                        nc.scalar.activation(out=aT, in_=pS, func=AF.Exp,
                                             bias=amaster[:, h, m:m + 1], scale=1.0)
                        if jt >= 4 * ic:
                            nc.gpsimd.affine_select(
                                out=aT, in_=aT, compare_op=mybir.AluOpType.is_ge,
                                fill=0.0, base=512 * ic - 128 * jt,
                                pattern=[[1, 512]], channel_multiplier=-1)
                        nc.tensor.matmul(pos[hp], v_T[:, h, jt, :], aT,
                                         start=(jt == 0), stop=(jt == njt - 1))
                for hp in range(2):
                    # free the PSUM bank first, normalize off-PSUM
                    otmp = p_norm.tile([65, 512], F32, tag="otmp")
                    nc.vector.tensor_copy(out=otmp, in_=pos[hp])
                    nc.vector.reciprocal(out=otmp[64:65, :],
                                         in_=otmp[64:65, :])
                    scr0 = p_norm.tile([1, 512], F32, tag="scr0")
                    nc.sync.dma_start(out=scr0, in_=otmp[64:65, :])
                    bc = p_norm.tile([64, 512], F32, tag="bc")
                    nc.gpsimd.partition_broadcast(bc[:, :], scr0[:, :])
                    if hp == 0:
                        nc.vector.tensor_tensor(
                            out=o_pair[0:64, hc, :], in0=otmp[0:64, :],
                            in1=bc[:, :], op=mybir.AluOpType.mult)
                    else:
                        scr3 = p_norm.tile([64, 512], F32R, tag="scr3")
                        nc.vector.tensor_tensor(
                            out=scr3, in0=otmp[0:64, :],
                            in1=bc[:, :], op=mybir.AluOpType.mult)
                        nc.sync.dma_start(out=o_pair[64:128, hc, :], in_=scr3)
            if debug:
                nc.sync.dma_start(out=aps["dbg_oP"][:, :, ic * 512:(ic + 1) * 512],
                                  in_=o_pair)
            for t in range(4):
                nt = 4 * ic + t
                for ec in range(2):
                    pmm = ps_mm.tile([128, 512], F32, tag="mm")
                    for hc in range(2):
                        nc.tensor.matmul(pmm, o_pair[:, hc, t * 128:(t + 1) * 128],
                                         wout[:, hc, ec * 512:(ec + 1) * 512],
                                         start=(hc == 0), stop=(hc == 1))
                    ot = p_out.tile([128, 512], F32, tag="ot")
                    nc.vector.tensor_copy(out=ot, in_=pmm)
                    nc.sync.dma_start(
                        out=aps["out_partial"][nt * 128:(nt + 1) * 128,
                                               ec * 512:(ec + 1) * 512],
                        in_=ot)

        if debug:
            nc.sync.dma_start(out=aps["dbg_q"], in_=q_aug)
            nc.sync.dma_start(out=aps["dbg_k"], in_=k_aug)


def _build(debug=False):
    nc = bacc.Bacc("TRN2", target_bir_lowering=False, debug=False, num_devices=8)
    aps = {}
    def din(name, shape, dt):
        aps[name] = nc.dram_tensor(name, shape, dt, kind="ExternalInput").ap()
    din("x_b", [N, DIM], F32)
    din("wqkv", [128, 8, 768], F32R)
    din("diag", [128, 3, 2, 7, 128], F32R)
    din("cbias", [128, 3, 2], F32)
    din("wbeta", [128, 3, 2], F32)
    din("brows", [2, 4, 2048], F32R)
    din("amaster", [128, 4, 16], F32)
    din("wout", [128, 2, 1024], F32R)
    aps["out_partial"] = nc.dram_tensor(
        "out_partial", [N, DIM], F32, kind="ExternalOutput").ap()
    if debug:
        for nm, shp in (("dbg_q", [128, 4, 2048]), ("dbg_k", [128, 4, 2048]),
                        ("dbg_v", [128, 2, 2048]), ("dbg_oP", [128, 2, 2048])):
            aps[nm] = nc.dram_tensor(nm, shp, F32R, kind="ExternalOutput").ap()
    with tile.TileContext(nc) as tc:
        _emit(tc, aps, debug)
    nc.compile()
    return nc


def _prepare_core(b, g, x, ln_gamma, ln_beta, w_qkv, dw, slopes, w_out):
    W_eff = w_qkv * ln_gamma[None, :]
    m = {}
    m["x_b"] = np.ascontiguousarray(x[b], dtype=np.float32)

    wqkv_sb = np.empty((128, 8, 768), np.float32)
    for prm in range(3):
        Wb = W_eff[prm * DIM + g * CG: prm * DIM + (g + 1) * CG]     # (256, 1024)
        wqkv_sb[:, :, prm * 256:(prm + 1) * 256] = (
            Wb.T.reshape(8, 128, 256).transpose(1, 0, 2))
    m["wqkv"] = np.ascontiguousarray(wqkv_sb)

    ksz = (0, 3, 5, 7)[g]
    w7 = np.zeros((3, 256, 7), np.float32)
    b7 = np.zeros((3, 256), np.float32)
    if ksz == 0:
        w7[:, :, 6] = 1.0
    else:
        for prm, p in enumerate("qkv"):
            w7[prm, :, 7 - ksz:] = dw[f"dw{p}_w{ksz}"][:, 0, :]
            b7[prm] = dw[f"dw{p}_b{ksz}"]
    w7[0] *= SCALE
    b7[0] *= SCALE

    dg = np.zeros((128, 3, 2, 7, 128), np.float32)
    pidx = np.arange(128)
    for prm in range(3):
        for cc in range(2):
            dg[pidx, prm, cc, :, pidx] = w7[prm, cc * 128: (cc + 1) * 128, :]
    m["diag"] = dg
    m["cbias"] = np.ascontiguousarray(
        b7.reshape(3, 2, 128).transpose(2, 0, 1))
    wb = np.stack([w_qkv[prm * DIM + g * CG: prm * DIM + (g + 1) * CG] @ ln_beta
                   for prm in range(3)])                             # (3, 256)
    m["wbeta"] = np.ascontiguousarray(
        wb.reshape(3, 2, 128).transpose(2, 0, 1)).astype(np.float32)

    sl = np.asarray(slopes, np.float64).reshape(-1)[4 * g: 4 * g + 4]
    Bm = -sl[:, None] * np.arange(512)[None, :]                      # (4, 512)
    B_hi = _round_f32r(Bm)
    B_lo = (Bm - B_hi).astype(np.float32)
    m["brows"] = np.ascontiguousarray(
        np.tile(np.stack([B_hi, B_lo]), (1, 1, 4)))                  # (2, 4, 2048)

    am = np.empty((128, 4, 16), np.float32)
    p = np.arange(128)[:, None]
    mm = np.arange(16)[None, :]
    for h in range(4):
        am[:, h, :] = sl[h] * (p + 128.0 * (mm - 12))
    m["amaster"] = am

    WoT = w_out[:, g * CG:(g + 1) * CG].T                            # (256, 1024)
    m["wout"] = np.ascontiguousarray(WoT.reshape(2, 128, 1024).transpose(1, 0, 2))
    return m


def kernel(**inputs):
    key = "dbg" if os.environ.get("KBENCH_DEBUG") else "main"
    if key not in _CACHE:
        _CACHE[key] = _build(debug=(key == "dbg"))
    nc = _CACHE[key]

    x = np.asarray(inputs["x"], np.float32)
    ln_gamma = np.asarray(inputs["ln_gamma"], np.float32)
    ln_beta = np.asarray(inputs["ln_beta"], np.float32)
    w_qkv = np.asarray(inputs["w_qkv"], np.float32)
    slopes = np.asarray(inputs["slopes"], np.float32)
    w_out = np.asarray(inputs["w_out"], np.float32)
    dw = {k: np.asarray(v, np.float32) for k, v in inputs.items()
          if k.startswith("dw")}

    in_maps = [
        _prepare_core(c // 4, c % 4, x, ln_gamma, ln_beta, w_qkv, dw, slopes, w_out)
        for c in range(8)
    ]
    res = run_bass_kernel_spmd(
        nc, in_maps, list(range(8)),
        trace=bool(os.environ.get("KBENCH_TRACE")))
    LAST["res"] = res
    outs = [r["out_partial"] for r in res.results]
    out = np.stack([outs[0] + outs[1] + outs[2] + outs[3],
                    outs[4] + outs[5] + outs[6] + outs[7]], axis=0)
    return out.astype(np.float32)


# revision 23
# speedup vs baseline: 1.8911x; 1.1242x over previous
"""Trainium2 Bass kernel for nn_CausalAttention_48945447305794.

Sharding: 8 cores = (batch b in {0,1}) x (head-group g in {0..3}).
Each core handles one batch and one group of 4 heads (one depthwise-conv
kernel size, padded to 7 taps), computes LN + QKV projection + causal
depthwise conv + alibi causal attention + a partial output projection
(its 256 channels of w_out). Host sums the 4 partial outputs per batch.

All matmuls use float32r (HW-verified: round-to-nearest 11-bit mantissa,
1 cycle/row at free-dim>=256) and keep contraction K>=96: the PE clock
gate (HAM) never warms past 1.2 GHz for K<=66 matmuls, so small-K
contractions are zero-padded to K=128.

Device math (per core):
  - LN over tokens (bn_stats); gamma folded into W_qkv on host; beta via
    per-partition add of W@beta on the raw qkv signal.
  - h transposed 128x128 via PE into h^T (d on partitions), f32r.
  - QKV = W-slices @ h^T; depthwise conv as 7 PSUM-accumulated
    diag(w_tap) matmuls; conv bias added during the PSUM->SBUF copy.
  - q/k stored head-major [128, head, n]: rows 0-63 head data, rows
    64-65 alibi aug rows (q side: -slope*fi hi/lo split on the f32r
    grid; k side: ones), rows 66-127 zeros. One K=128 matmul per S^T
    tile [j=128, i=512] then computes q.k + alibi column term exactly.
  - Alibi row term slope*(j-512ic) enters as the per-partition bias of
    the Exp activation; the per-query remainder cancels in softmax
    normalization. Causal mask: gpsimd affine_select zero-fill on
    diagonal-crossing tiles after exp.
  - O^T = v_plus.T @ A^T with a ones column appended to v giving the
    softmax denominator as PSUM row 64; PSUM copied out immediately,
    normalized off-PSUM via reciprocal + partition_broadcast + multiply.
  - Out-projection with heads paired for K=128 matmuls.
"""

import os
import sys

for _p in ("/opt/trn_rl_repo",):
    if _p not in sys.path and os.path.isdir(_p):
        sys.path.insert(0, _p)

import numpy as np

import concourse.bass as bass
import concourse.mybir as mybir
import concourse.tile as tile
from concourse import bacc
from concourse.bass_utils import run_bass_kernel_spmd

F32 = mybir.dt.float32
F32R = mybir.dt.float32r
U32 = mybir.dt.uint32
AF = mybir.ActivationFunctionType

B, N, DIM = 2, 2048, 1024
HEADS, DH = 16, 64
G, HPG, CG = 4, 4, 256
NT = N // 128          # 16 token tiles
NIC = N // 512         # 4 i-chunks
SCALE = DH ** -0.5
ONES_F32_BITS = 0x3F800000

_CACHE = {}
LAST = {}


def _round_f32r(a):
    """Round to nearest on the f32r grid (11-bit mantissa), HW-verified."""
    a = np.asarray(a, np.float64)
    out = np.zeros_like(a)
    nz = a != 0
    s = np.exp2(np.floor(np.log2(np.abs(a[nz]))))
    out[nz] = np.round(a[nz] / s * 2048.0) * s / 2048.0
    return out.astype(np.float32)


def _emit(tc, aps, debug):
    nc = tc.nc
    import contextlib
    ctx = contextlib.ExitStack()
    with ctx:
        singles = ctx.enter_context(tc.tile_pool(name="singles", bufs=1))
        p_w = ctx.enter_context(tc.tile_pool(name="p_w", bufs=1))
        p_diag = ctx.enter_context(tc.tile_pool(name="p_diag", bufs=1))
        p_raw = ctx.enter_context(tc.tile_pool(name="p_raw", bufs=1))
        p_qkv = ctx.enter_context(tc.tile_pool(name="p_qkv", bufs=1))
        p_v = ctx.enter_context(tc.tile_pool(name="p_v", bufs=1))
        p_vt = ctx.enter_context(tc.tile_pool(name="p_vt", bufs=1))
        p_oT = ctx.enter_context(tc.tile_pool(name="p_oT", bufs=1))
        p_x = ctx.enter_context(tc.tile_pool(name="p_x", bufs=2))
        p_h = ctx.enter_context(tc.tile_pool(name="p_h", bufs=1))
        p_hT = ctx.enter_context(tc.tile_pool(name="p_hT", bufs=1))
        p_aT = ctx.enter_context(tc.tile_pool(name="p_aT", bufs=3))
        p_norm = ctx.enter_context(tc.tile_pool(name="p_norm", bufs=1))
        p_scr = ctx.enter_context(tc.tile_pool(name="p_scr", bufs=2))
        p_out = ctx.enter_context(tc.tile_pool(name="p_out", bufs=2))
        p_stat = ctx.enter_context(tc.tile_pool(name="p_stat", bufs=4))

        ps_tr = ctx.enter_context(tc.tile_pool(name="ps_tr", bufs=1, space="PSUM"))
        ps_mm = ctx.enter_context(tc.tile_pool(name="ps_mm", bufs=2, space="PSUM"))
        ps_s = ctx.enter_context(tc.tile_pool(name="ps_s", bufs=3, space="PSUM"))
        ps_o = ctx.enter_context(tc.tile_pool(name="ps_o", bufs=1, space="PSUM"))

        # ---- constants / small inputs
        identr = singles.tile([128, 128], F32R)
        nc.gpsimd.memset(identr.bitcast(U32), 0)
        nc.gpsimd.affine_select(
            out=identr, in_=identr, compare_op=mybir.AluOpType.not_equal,
            fill=1.0, base=0, pattern=[[-1, 128]], channel_multiplier=1)
        eps_t = singles.tile([128, 1], F32)
        nc.vector.memset(eps_t, 1e-5)
        cbias = singles.tile([128, 3, 2], F32)
        nc.sync.dma_start(out=cbias, in_=aps["cbias"])
        wbeta = singles.tile([128, 3, 2], F32)
        nc.sync.dma_start(out=wbeta, in_=aps["wbeta"])
        amaster = singles.tile([128, 4, 16], F32)
        nc.sync.dma_start(out=amaster, in_=aps["amaster"])

        wqkv = p_w.tile([128, 8, 768], F32R)
        for kd in range(8):
            nc.gpsimd.dma_start(out=wqkv[:, kd, :], in_=aps["wqkv"][:, kd, :])
        diag = p_diag.tile([128, 3, 2, 7, 128], F32R, tag="diag")
        for prm in range(3):
            nc.gpsimd.dma_start(out=diag[:, prm], in_=aps["diag"][:, prm])

        raws = [p_raw.tile([128, 2, 520], F32R, tag=f"raw{i}", name=f"raw{i}")
                for i in range(3)]
        for r in raws:
            nc.vector.memset(r[:, :, 0:8].bitcast(U32), 0)

        # q/k head-major with aug rows; rows 66-127 must be zero
        q_aug = p_qkv.tile([128, 4, 2048], F32R, tag="q_aug")
        k_aug = p_qkv.tile([128, 4, 2048], F32R, tag="k_aug")
        nc.gpsimd.memset(q_aug[64:128].bitcast(U32), 0)
        nc.gpsimd.memset(k_aug[64:128].bitcast(U32), 0)
        nc.sync.dma_start(out=q_aug[64:66], in_=aps["brows"])
        nc.gpsimd.memset(k_aug[64:66].bitcast(U32), ONES_F32_BITS)

        v_T = p_vt.tile([128, 4, 16, 65], F32R)
        nc.gpsimd.memset(v_T[:, :, :, 64:65].bitcast(U32), ONES_F32_BITS)

        # ---- phase 1+2: LN + transpose + QKV + conv + v-transpose, per chunk
        for ic in range(NIC):
            hT = p_hT.tile([128, 8, 512], F32R, tag="hT")
            for t in range(4):
                nt = 4 * ic + t
                xt = p_x.tile([128, DIM], F32, tag="x")
                nc.sync.dma_start(out=xt, in_=aps["x_b"][nt * 128:(nt + 1) * 128, :])
                stats = p_stat.tile([128, 2, 6], F32, tag="stats")
                xg = xt.rearrange("p (s d) -> p s d", s=2)
                for sgi in range(2):
                    nc.vector.bn_stats(out=stats[:, sgi, :], in_=xg[:, sgi, :])
                mv = p_stat.tile([128, 2], F32, tag="mv")
                nc.vector.bn_aggr(out=mv, in_=stats)
                nc.scalar.activation(out=mv[:, 1:2], in_=mv[:, 1:2], func=AF.Sqrt,
                                     bias=eps_t, scale=1.0)
                nc.vector.reciprocal(out=mv[:, 1:2], in_=mv[:, 1:2])
                ht = p_h.tile([128, DIM], F32R, tag="h")
                nc.vector.tensor_scalar(
                    out=ht, in0=xt, scalar1=mv[:, 0:1], scalar2=mv[:, 1:2],
                    op0=mybir.AluOpType.subtract, op1=mybir.AluOpType.mult)
                for kq in range(2):
                    ptr = ps_tr.tile([128, 4, 128], F32R, tag="tr")
                    for kk in range(4):
                        kd = 4 * kq + kk
                        nc.tensor.transpose(ptr[:, kk, :],
                                            ht[:, kd * 128:(kd + 1) * 128], identr)
                    nc.vector.tensor_copy(
                        out=hT[:, 4 * kq:4 * kq + 4, t * 128:(t + 1) * 128],
                        in_=ptr)
            vc = p_v.tile([128, 2, 512], F32R, tag="vc")
            for prm in range(3):
                for cc in range(2):
                    pmm = ps_mm.tile([128, 512], F32, tag="mm")
                    cbase = prm * 256 + cc * 128
                    for kd in range(8):
                        nc.tensor.matmul(pmm, wqkv[:, kd, cbase:cbase + 128],
                                         hT[:, kd, :], start=(kd == 0), stop=(kd == 7))
                    nc.scalar.activation(
                        out=raws[prm][:, cc, 8:520], in_=pmm, func=AF.Identity,
                        bias=wbeta[:, prm, cc:cc + 1], scale=1.0)
                for cc in range(2):
                    pmm = ps_mm.tile([128, 512], F32, tag="mm")
                    for s in range(7):
                        nc.tensor.matmul(pmm, diag[:, prm, cc, s, :],
                                         raws[prm][:, cc, 2 + s: 2 + s + 512],
                                         start=(s == 0), stop=(s == 6))
                    isl = slice(ic * 512, (ic + 1) * 512)
                    if prm < 2:
                        dest = (q_aug, k_aug)[prm]
                        # even head of the pair: aligned copy
                        nc.vector.tensor_scalar(
                            out=dest[0:64, 2 * cc, isl], in0=pmm[0:64, :],
                            scalar1=cbias[0:64, prm, cc:cc + 1], scalar2=None,
                            op0=mybir.AluOpType.add)
                        # odd head: bias-add into scratch, DMA partition-shift
                        scrA = p_scr.tile([128, 512], F32R, tag="scrA")
                        nc.vector.tensor_scalar(
                            out=scrA[64:128, :], in0=pmm[64:128, :],
                            scalar1=cbias[64:128, prm, cc:cc + 1], scalar2=None,
                            op0=mybir.AluOpType.add)
                        nc.sync.dma_start(out=dest[0:64, 2 * cc + 1, isl],
                                          in_=scrA[64:128, :])
                    else:
                        nc.vector.tensor_scalar(
                            out=vc[:, cc, :],
                            in0=pmm, scalar1=cbias[:, prm, cc:cc + 1], scalar2=None,
                            op0=mybir.AluOpType.add)
                # roll the 8-column tail for the next chunk's left taps
                tail = p_stat.tile([128, 2, 8], F32R, tag="tail")
                nc.vector.tensor_copy(out=tail, in_=raws[prm][:, :, 512:520])
                nc.vector.tensor_copy(out=raws[prm][:, :, 0:8], in_=tail)
            # v transposes for this chunk
            for cc in range(2):
                for t in range(4):
                    jt = 4 * ic + t
                    ptr = ps_tr.tile([128, 128], F32R, tag="tr")
                    nc.tensor.transpose(ptr, vc[:, cc, t * 128:(t + 1) * 128], identr)
                    nc.vector.tensor_copy(out=v_T[:, 2 * cc, jt, 0:64],
                                          in_=ptr[:, 0:64])
                    nc.vector.tensor_copy(out=v_T[:, 2 * cc + 1, jt, 0:64],
                                          in_=ptr[:, 64:128])
            if debug:
                nc.sync.dma_start(out=aps["dbg_v"][:, :, ic * 512:(ic + 1) * 512],
                                  in_=vc)

        # ---- phase 4+5: attention + partial out-projection, per i-chunk
        wout = p_w.tile([128, 2, 1024], F32R)
        nc.gpsimd.dma_start(out=wout, in_=aps["wout"])
        for ic in range(NIC):
            o_pair = p_oT.tile([128, 2, 512], F32R, tag="oP")
            njt = 4 * ic + 4              # j tiles 0 .. 4ic+3
            for hc in range(2):
                pos = [ps_o.tile([65, 512], F32, tag=f"po{i}", name=f"po{i}")
                       for i in range(2)]
                for jt in range(njt):
                    for hp in range(2):
                        h = 2 * hc + hp
                        pS = ps_s.tile([128, 512], F32, tag="pS", name="pS")
                        nc.tensor.matmul(
                            pS,
                            k_aug[:, h, jt * 128:(jt + 1) * 128],
                            q_aug[:, h, ic * 512:(ic + 1) * 512],
                            start=True, stop=True)
                        aT = p_aT.tile([128, 512], F32R, tag="aT")
                        m = jt - 4 * ic + 12
                        nc.scalar.activation(out=aT, in_=pS, func=AF.Exp,
                                             bias=amaster[:, h, m:m + 1], scale=1.0)
                        if jt >= 4 * ic:
                            nc.gpsimd.affine_select(
                                out=aT, in_=aT, compare_op=mybir.AluOpType.is_ge,
                                fill=0.0, base=512 * ic - 128 * jt,
                                pattern=[[1, 512]], channel_multiplier=-1)
                        nc.tensor.matmul(pos[hp], v_T[:, h, jt, :], aT,
                                         start=(jt == 0), stop=(jt == njt - 1))
                for hp in range(2):
                    # free the PSUM bank first, normalize off-PSUM
                    otmp = p_norm.tile([65, 512], F32, tag="otmp")
                    nc.vector.tensor_copy(out=otmp, in_=pos[hp])
                    scr0 = p_norm.tile([1, 512], F32, tag="scr0")
                    nc.sync.dma_start(out=scr0, in_=otmp[64:65, :])
                    nc.vector.reciprocal_approx_fast(out=scr0, in_=scr0)
                    bc = p_norm.tile([64, 512], F32, tag="bc")
                    nc.gpsimd.partition_broadcast(bc[:, :], scr0[:, :])
                    if hp == 0:
                        nc.vector.tensor_tensor(
                            out=o_pair[0:64, hc, :], in0=otmp[0:64, :],
                            in1=bc[:, :], op=mybir.AluOpType.mult)
                    else:
                        scr3 = p_norm.tile([64, 512], F32R, tag="scr3")
                        nc.vector.tensor_tensor(
                            out=scr3, in0=otmp[0:64, :],
                            in1=bc[:, :], op=mybir.AluOpType.mult)
                        nc.sync.dma_start(out=o_pair[64:128, hc, :], in_=scr3)
            if debug:
                nc.sync.dma_start(out=aps["dbg_oP"][:, :, ic * 512:(ic + 1) * 512],
                                  in_=o_pair)
            for t in range(4):
                nt = 4 * ic + t
                for ec in range(2):
                    pmm = ps_mm.tile([128, 512], F32, tag="mm")
                    for hc in range(2):
                        nc.tensor.matmul(pmm, o_pair[:, hc, t * 128:(t + 1) * 128],
                                         wout[:, hc, ec * 512:(ec + 1) * 512],
                                         start=(hc == 0), stop=(hc == 1))
                    ot = p_out.tile([128, 512], F32, tag="ot")
                    nc.vector.tensor_copy(out=ot, in_=pmm)
                    nc.sync.dma_start(
                        out=aps["out_partial"][nt * 128:(nt + 1) * 128,
                                               ec * 512:(ec + 1) * 512],
                        in_=ot)

        if debug:
            nc.sync.dma_start(out=aps["dbg_q"], in_=q_aug)
            nc.sync.dma_start(out=aps["dbg_k"], in_=k_aug)


def _build(debug=False):
    nc = bacc.Bacc("TRN2", target_bir_lowering=False, debug=False, num_devices=8)
    aps = {}
    def din(name, shape, dt):
        aps[name] = nc.dram_tensor(name, shape, dt, kind="ExternalInput").ap()
    din("x_b", [N, DIM], F32)
    din("wqkv", [128, 8, 768], F32R)
    din("diag", [128, 3, 2, 7, 128], F32R)
    din("cbias", [128, 3, 2], F32)
    din("wbeta", [128, 3, 2], F32)
    din("brows", [2, 4, 2048], F32R)
    din("amaster", [128, 4, 16], F32)
    din("wout", [128, 2, 1024], F32R)
    aps["out_partial"] = nc.dram_tensor(
        "out_partial", [N, DIM], F32, kind="ExternalOutput").ap()
    if debug:
        for nm, shp in (("dbg_q", [128, 4, 2048]), ("dbg_k", [128, 4, 2048]),
                        ("dbg_v", [128, 2, 2048]), ("dbg_oP", [128, 2, 2048])):
            aps[nm] = nc.dram_tensor(nm, shp, F32R, kind="ExternalOutput").ap()
    with tile.TileContext(nc) as tc:
        _emit(tc, aps, debug)
    nc.compile()
    return nc


def _prepare_core(b, g, x, ln_gamma, ln_beta, w_qkv, dw, slopes, w_out):
    W_eff = w_qkv * ln_gamma[None, :]
    m = {}
    m["x_b"] = np.ascontiguousarray(x[b], dtype=np.float32)

    wqkv_sb = np.empty((128, 8, 768), np.float32)
    for prm in range(3):
        Wb = W_eff[prm * DIM + g * CG: prm * DIM + (g + 1) * CG]     # (256, 1024)
        wqkv_sb[:, :, prm * 256:(prm + 1) * 256] = (
            Wb.T.reshape(8, 128, 256).transpose(1, 0, 2))
    m["wqkv"] = np.ascontiguousarray(wqkv_sb)

    ksz = (0, 3, 5, 7)[g]
    w7 = np.zeros((3, 256, 7), np.float32)
    b7 = np.zeros((3, 256), np.float32)
    if ksz == 0:
        w7[:, :, 6] = 1.0
    else:
        for prm, p in enumerate("qkv"):
            w7[prm, :, 7 - ksz:] = dw[f"dw{p}_w{ksz}"][:, 0, :]
            b7[prm] = dw[f"dw{p}_b{ksz}"]
    w7[0] *= SCALE
    b7[0] *= SCALE

    dg = np.zeros((128, 3, 2, 7, 128), np.float32)
    pidx = np.arange(128)
    for prm in range(3):
        for cc in range(2):
            dg[pidx, prm, cc, :, pidx] = w7[prm, cc * 128: (cc + 1) * 128, :]
    m["diag"] = dg
    m["cbias"] = np.ascontiguousarray(
        b7.reshape(3, 2, 128).transpose(2, 0, 1))
    wb = np.stack([w_qkv[prm * DIM + g * CG: prm * DIM + (g + 1) * CG] @ ln_beta
                   for prm in range(3)])                             # (3, 256)
    m["wbeta"] = np.ascontiguousarray(
        wb.reshape(3, 2, 128).transpose(2, 0, 1)).astype(np.float32)

    sl = np.asarray(slopes, np.float64).reshape(-1)[4 * g: 4 * g + 4]
    Bm = -sl[:, None] * np.arange(512)[None, :]                      # (4, 512)
    B_hi = _round_f32r(Bm)
    B_lo = (Bm - B_hi).astype(np.float32)
    m["brows"] = np.ascontiguousarray(
        np.tile(np.stack([B_hi, B_lo]), (1, 1, 4)))                  # (2, 4, 2048)

    am = np.empty((128, 4, 16), np.float32)
    p = np.arange(128)[:, None]
    mm = np.arange(16)[None, :]
    for h in range(4):
        am[:, h, :] = sl[h] * (p + 128.0 * (mm - 12))
    m["amaster"] = am

    WoT = w_out[:, g * CG:(g + 1) * CG].T                            # (256, 1024)
    m["wout"] = np.ascontiguousarray(WoT.reshape(2, 128, 1024).transpose(1, 0, 2))
    return m


def kernel(**inputs):
    key = "dbg" if os.environ.get("KBENCH_DEBUG") else "main"
    if key not in _CACHE:
        _CACHE[key] = _build(debug=(key == "dbg"))
    nc = _CACHE[key]

    x = np.asarray(inputs["x"], np.float32)
    ln_gamma = np.asarray(inputs["ln_gamma"], np.float32)
    ln_beta = np.asarray(inputs["ln_beta"], np.float32)
    w_qkv = np.asarray(inputs["w_qkv"], np.float32)
    slopes = np.asarray(inputs["slopes"], np.float32)
    w_out = np.asarray(inputs["w_out"], np.float32)
    dw = {k: np.asarray(v, np.float32) for k, v in inputs.items()
          if k.startswith("dw")}

    in_maps = [
        _prepare_core(c // 4, c % 4, x, ln_gamma, ln_beta, w_qkv, dw, slopes, w_out)
        for c in range(8)
    ]
    res = run_bass_kernel_spmd(
        nc, in_maps, list(range(8)),
        trace=bool(os.environ.get("KBENCH_TRACE")))
    LAST["res"] = res
    outs = [r["out_partial"] for r in res.results]
    out = np.stack([outs[0] + outs[1] + outs[2] + outs[3],
                    outs[4] + outs[5] + outs[6] + outs[7]], axis=0)
    return out.astype(np.float32)
